# revision 33
# baseline (speedup 1.0000x reference)
"""Trainium2 Bass kernel for single-token decode attention (NaiveAttention).

Math (per reference):
  q = x @ W_Q.T ; k_new = x @ W_K.T ; v_new = x @ W_V.T        (each (32, 128))
  k_cache[seq, pos] = k_new ; v_cache[seq, pos] = v_new
  K = k_cache[seq, :pos+1] ; V = v_cache[seq, :pos+1]
  scores = (q . K) / sqrt(128) ; attn = softmax(scores)
  out = (attn . V) @ W_O.T                                     ((1, 1, 4096))

Sharding: tensor-parallel over heads. 8 cores x 4 heads. W_Q/W_K/W_V are
sharded column-wise (after transpose), W_O row-wise; each core computes a
partial (4096,) output vector and the host sums the 8 partials.

The kernel is DMA-bandwidth-bound: per core it must stream 4 weight shards
(W_Q/W_K/W_V/W_O) + its 4 heads of K and V, all exactly once. Everything is
cast to bf16 on the host (24 MiB/core instead of 48 MiB fp32; measured
~360 GB/s per-core HBM => ~70 us floor) and laid out in DRAM to match the
SBUF destination exactly, so every DMA is fully contiguous. All large
matmuls run bf16 (1 PE cycle/column, fp32 PSUM accumulation).

Scores are computed directly in column form: per (head, 128-seq tile) the
K tile (dk=128 partitions x 128 seq) is the stationary operand and the q
column is the moving operand, so exp() on the scalar engine writes attn
columns straight into SBUF (p_all) with no PE transposes or DVE copies.
A@V is head-batched: lhsT = p(128s x 4heads), rhs = [V_h0|..|V_h3]
(128s x 512); the diagonal 128-blocks of the (4,512) result are the
per-head outputs. The cache slot at s = 4095 is stale: the host zeroes
that K column and V row, so its prob is exactly exp(0) = 1 (subtracted
from the softmax denominator) and its A@V term is exactly 0; the true
k_new/v_new contribution enters as a rank-1 PSUM update plus exp(q.k_new)
on the denominator.
"""

import sys

if "/opt/trn_rl_repo" not in sys.path:
    sys.path.insert(0, "/opt/trn_rl_repo")

import numpy as np
import ml_dtypes

BF16 = ml_dtypes.bfloat16

D_MODEL = 4096
N_HEADS = 32
D_K = 128
S = 4096          # pos + 1 for the compiled fast path
N_CORES = 8
HPC = N_HEADS // N_CORES          # heads per core = 4
MPC = HPC * D_K                   # model dims per core = 512
NT = S // 128                     # 32 seq tiles
NC = S // 512                     # 8 512-wide chunks
INV_SQRT_DK = 1.0 / float(np.sqrt(D_K))

_CACHE = {}


def _build_program():
    """Build + compile the per-core Bass program (identical on all cores)."""
    if "nc" in _CACHE:
        return _CACHE["nc"]

    from concourse import bacc, mybir
    import concourse.tile as tile
    from concourse.masks import make_identity

    f32 = mybir.dt.float32
    bf16 = mybir.dt.bfloat16
    AF = mybir.ActivationFunctionType
    ALU = mybir.AluOpType
    AX = mybir.AxisListType

    nc = bacc.Bacc("TRN2", target_bir_lowering=False, debug=False,
                   num_devices=N_CORES)

    xt_d = nc.dram_tensor("xt", [128, NT], bf16, kind="ExternalInput")
    wq_d = nc.dram_tensor("wq", [128, NT * MPC], bf16, kind="ExternalInput")
    wk_d = nc.dram_tensor("wk", [128, NT * MPC], bf16, kind="ExternalInput")
    wv_d = nc.dram_tensor("wv", [128, NT * MPC], bf16, kind="ExternalInput")
    wo_d = nc.dram_tensor("wo", [128, NC * HPC * MPC], bf16,
                          kind="ExternalInput")          # (128, 8*4*512)
    kt_d = nc.dram_tensor("kt", [128, NC * HPC * MPC], bf16,
                          kind="ExternalInput")          # (128, 8*4*512)
    v_d = nc.dram_tensor("v", [128, NT * HPC * D_K], bf16,
                         kind="ExternalInput")           # (128, 32*4*128)
    out_d = nc.dram_tensor("out", [1, D_MODEL], f32, kind="ExternalOutput")

    with tile.TileContext(nc) as tc:
        with (
            tc.tile_pool(name="singles", bufs=1) as singles,
            tc.tile_pool(name="wpool", bufs=4) as wpool,
            tc.tile_pool(name="projp", bufs=3, space="PSUM") as projp,
            tc.tile_pool(name="scolp", bufs=2, space="PSUM") as scolp,
            tc.tile_pool(name="tp", bufs=2, space="PSUM") as tp,
            tc.tile_pool(name="av4p", bufs=1, space="PSUM") as av4p,
        ):
            # ---- constants / persistent tiles ----
            xt = singles.tile([128, NT], bf16, tag="xt")
            nc.sync.dma_start(xt[:], xt_d.ap())
            ident = singles.tile([128, 128], f32, tag="ident")
            make_identity(nc, ident[:])
            ones_col = singles.tile([128, 1], f32, tag="ones_col")
            nc.vector.memset(ones_col[:], 1.0)

            ktile = singles.tile([128, NC, HPC, MPC], bf16, tag="ktile")
            v4 = singles.tile([128, NT * HPC * D_K], bf16, tag="v4")
            p_all = singles.tile([128, HPC, NT], bf16, tag="p_all")
            qsb = singles.tile([128, HPC], bf16, tag="qsb")
            ksb = singles.tile([128, HPC], bf16, tag="ksb")
            qrow = singles.tile([1, MPC], f32, tag="qrow")
            krow = singles.tile([1, MPC], f32, tag="krow")
            vrow = singles.tile([1, MPC], bf16, tag="vrow")
            rsum = singles.tile([128, HPC], f32, tag="rsum")
            setot = singles.tile([1, HPC], f32, tag="setot")
            recrow = singles.tile([1, HPC], f32, tag="recrow")
            rec4 = singles.tile([HPC, 1], f32, tag="rec4")
            p49f = singles.tile([1, HPC], f32, tag="p49f")
            p49b = singles.tile([1, HPC], bf16, tag="p49b")
            av4n = singles.tile([HPC, MPC], f32, tag="av4n")
            avn = singles.tile([128, HPC], bf16, tag="avn")
            orow = singles.tile([1, D_MODEL], f32, tag="orow")
            wv_tiles = [singles.tile([128, 8, MPC], bf16, name=f"wvt{b}",
                                     tag=f"wvt{b}") for b in range(4)]
            wo_tiles = [singles.tile([128, 2, HPC, MPC], bf16, name=f"wot{b}",
                                     tag=f"wot{b}") for b in range(4)]

            def emit_proj(w_dram, tiles=None):
                """acc(1,512) = x^T @ W_chunk, W streamed as 4 x 1MiB.

                tiles: dedicated SBUF tiles (DMA never throttled by PE
                consumption); falls back to the shared wpool.
                """
                w_ap = w_dram.ap().rearrange("p (b c m) -> b p c m", b=4, c=8)
                acc = projp.tile([1, MPC], f32, tag="projp")
                wts = []
                for b in range(4):
                    wt = (tiles[b] if tiles is not None
                          else wpool.tile([128, 8, MPC], bf16, tag="wt"))
                    nc.sync.dma_start(wt[:], w_ap[b])
                    wts.append(wt)
                for b in range(4):
                    for c in range(8):
                        t = b * 8 + c
                        nc.tensor.matmul(
                            acc[:], xt[:, t:t + 1], wts[b][:, c, :],
                            start=(t == 0), stop=(t == NT - 1),
                            skip_group_check=True)
                return acc

            def row_to_cols(row_sb, dst_sb):
                """(1,512) f32 row -> (128,4) bf16 columns via 4 transposes."""
                t4 = tp.tile([128, HPC], f32, tag="tp")
                for i in range(HPC):
                    nc.tensor.matmul(t4[:, i:i + 1],
                                     row_sb[0:1, i * 128:(i + 1) * 128],
                                     ident[0:1, 0:1], is_transpose=True,
                                     skip_group_check=True)
                nc.vector.tensor_copy(dst_sb[:], t4[:])

            # ---- phase 1: W_Q stream -> q; q -> columns ----
            q_acc = emit_proj(wq_d)
            nc.vector.tensor_copy(qrow[:], q_acc[:])
            row_to_cols(qrow, qsb)

            # ---- phase 2: K stream (8 x 0.5MiB, chunk-major) + scores ----
            # per (seq-tile, head): stationary = K tile (dk x 128s), moving =
            # q column -> score COLUMN in PSUM; exp writes p_all directly.
            kt_ap = kt_d.ap().rearrange("p (c h m) -> c p h m", c=NC, h=HPC)
            for c in range(NC):
                nc.sync.dma_start(ktile[:, c, :, :], kt_ap[c])

            for c in range(NC):
                for j in range(4):
                    t = 4 * c + j
                    scol = scolp.tile([128, HPC], f32, tag="scolp")
                    for g in range(HPC):
                        nc.tensor.matmul(
                            scol[:, g:g + 1],
                            ktile[:, c, g, j * 128:(j + 1) * 128],
                            qsb[:, g:g + 1], skip_group_check=True)
                    nc.scalar.activation(p_all[:, :, t], scol[:], AF.Exp,
                                         scale=INV_SQRT_DK)

            # softmax denominator: sum p over seq (reduce tiles, then a
            # ones-matmul over partitions). The stale s=4095 slot contributes
            # exactly exp(0) = 1 (host zeroed that K column): subtract it.
            nc.vector.tensor_reduce(rsum[:], p_all[:], axis=AX.X, op=ALU.add)
            serow = tp.tile([1, HPC], f32, tag="tp")
            nc.tensor.matmul(serow[:], ones_col[:], rsum[:],
                             skip_group_check=True)

            # ---- phase 4: W_K / W_V streams -> k_new, v_new ----
            k_acc = emit_proj(wk_d)
            nc.vector.tensor_copy(krow[:], k_acc[:])
            row_to_cols(krow, ksb)

            # p4095_h = exp(q_h . k_new_h / sqrt(dk)); sumexp finalization
            sc4 = tp.tile([1, HPC], f32, tag="tp")
            for h in range(HPC):
                nc.tensor.matmul(sc4[:, h:h + 1], ksb[:, h:h + 1],
                                 qsb[:, h:h + 1], skip_group_check=True)
            nc.scalar.activation(p49f[:], sc4[:], AF.Exp, scale=INV_SQRT_DK)
            nc.vector.tensor_copy(p49b[:], p49f[:])
            nc.vector.tensor_scalar_add(setot[:], p49f[:], -1.0)
            nc.vector.tensor_add(setot[:], setot[:], serow[:])
            nc.vector.reciprocal(recrow[:], setot[:])

            v_acc = emit_proj(wv_d, tiles=wv_tiles)
            nc.vector.tensor_copy(vrow[:], v_acc[:])

            # rec (1,4) row -> (4,1) per-partition scalars
            rect = tp.tile([HPC, 1], f32, tag="tp")
            nc.tensor.matmul(rect[:], recrow[:], ident[0:1, 0:1],
                             is_transpose=True, skip_group_check=True)
            nc.vector.tensor_copy(rec4[:], rect[:])

            # ---- phase 5: V stream (4 x 1MiB) + head-batched A@V ----
            # emitted after the W_V projection so the A@V matmuls track the
            # V DMA and the rank-1 closer can follow immediately
            # last piece split in two so its completion semaphore (which
            # gates the final A@V matmuls) fires earlier
            v_ap = v_d.ap().rearrange("p (b r) -> b p r", b=8)
            for b in range(8):
                if b < 6 and b % 2 == 0:
                    nc.sync.dma_start(
                        v4[:, b * 2048:(b + 2) * 2048],
                        v_d.ap()[:, b * 2048:(b + 2) * 2048])
                elif b >= 6:
                    nc.sync.dma_start(v4[:, b * 2048:(b + 1) * 2048], v_ap[b])

            av4 = av4p.tile([HPC, HPC * D_K], f32, tag="av4")
            for t in range(NT):
                nc.tensor.matmul(av4[:], p_all[:, :, t],
                                 v4[:, t * 512:(t + 1) * 512],
                                 start=(t == 0), stop=False,
                                 skip_group_check=True)
            # av4[g, (h,d)] += p4095_g * v_new_h[d]; diagonal g==h is the
            # true s = S-1 contribution; closes the A@V accumulation group
            nc.tensor.matmul(av4[:], p49b[:], vrow[:],
                             start=False, stop=True, skip_group_check=True)

            # ---- phase 6: normalize + diagonal extraction ----
            nc.vector.tensor_scalar_mul(av4n[:], av4[:], rec4[:, 0:1])
            for g in range(HPC):
                avt = tp.tile([128, HPC], f32, tag="tp")
                nc.tensor.matmul(avt[:], av4n[0:HPC, g * 128:(g + 1) * 128],
                                 ident[0:HPC, 0:HPC], is_transpose=True,
                                 skip_group_check=True)
                nc.vector.tensor_copy(avn[:, g:g + 1], avt[:, g:g + 1])

            # ---- phase 7: W_O stream + partial output ----
            # all W_O piece DMAs enqueue before the output-chunk DMAs so the
            # sync queue never blocks the stream on the compute chain
            wo_ap = wo_d.ap().rearrange("p (b j h m) -> b p j h m",
                                        b=4, j=2, h=HPC)
            for b in range(3):
                nc.sync.dma_start(wo_tiles[b][:], wo_ap[b])
            for jj in range(2):     # last piece split: earlier completion
                nc.sync.dma_start(wo_tiles[3][:, jj], wo_ap[3][:, jj])
            for b in range(4):
                wt = wo_tiles[b]
                for jj in range(2):
                    jc = b * 2 + jj
                    wo_ps = projp.tile([1, MPC], f32, tag="projp")
                    for h in range(HPC):
                        nc.tensor.matmul(
                            wo_ps[:], avn[:, h:h + 1], wt[:, jj, h, :],
                            start=(h == 0), stop=(h == HPC - 1),
                            skip_group_check=True)
                    nc.vector.tensor_copy(orow[:, jc * 512:(jc + 1) * 512],
                                          wo_ps[:])
            # single 16 KiB result DMA: avoids 8 serial sync-engine issues
            nc.sync.dma_start(out_d.ap(), orow[:])

    nc.compile()
    _CACHE["nc"] = nc
    return nc


def _numpy_reference(x, seq, pos, k_cache, v_cache, W_Q, W_K, W_V, W_O):
    """Fallback for shapes the compiled program doesn't cover."""
    xf = x.reshape(-1).astype(np.float32)
    q = (W_Q @ xf).reshape(N_HEADS, D_K)
    k_new = (W_K @ xf).reshape(N_HEADS, D_K)
    v_new = (W_V @ xf).reshape(N_HEADS, D_K)
    K = np.array(k_cache[seq, :pos + 1], dtype=np.float32)
    V = np.array(v_cache[seq, :pos + 1], dtype=np.float32)
    K[pos] = k_new
    V[pos] = v_new
    scores = np.einsum("hd,shd->hs", q, K) / np.float32(np.sqrt(D_K))
    scores -= scores.max(axis=-1, keepdims=True)
    e = np.exp(scores)
    attn = e / e.sum(axis=-1, keepdims=True)
    out = np.einsum("hs,shd->hd", attn, V).reshape(-1)
    return (W_O @ out).reshape(1, 1, D_MODEL).astype(np.float32)


def _make_in_maps(x, seq, k_cache, v_cache, W_Q, W_K, W_V, W_O):
    xt = np.ascontiguousarray(x.reshape(NT, 128).T).astype(BF16)
    k_seq = np.asarray(k_cache[seq], dtype=np.float32)   # (S, H, dk)
    v_seq = np.asarray(v_cache[seq], dtype=np.float32)

    def wproj_layout(W_shard):
        # (512, 4096) -> (128, 32*512): [p, t, m] = W_shard[m, t*128+p]
        return np.ascontiguousarray(
            W_shard.T.reshape(NT, 128, MPC).transpose(1, 0, 2)
            .reshape(128, NT * MPC)).astype(BF16)

    in_maps = []
    for c in range(N_CORES):
        sl = slice(c * MPC, (c + 1) * MPC)
        hs = slice(c * HPC, (c + 1) * HPC)
        # W_O[:, sl] -> (128, 8, 4, 512): [p, jc, h, m] = W_O[jc*512+m, sl0+h*128+p]
        wo = (W_O[:, sl].reshape(NC, MPC, HPC, 128)
              .transpose(3, 0, 2, 1).reshape(128, NC * HPC * MPC))
        # K -> (128, 8, 4, 512): [d, c8, h, j] = K[c8*512+j, h, d]
        kt = np.ascontiguousarray(
            k_seq[:, hs, :].reshape(NC, MPC, HPC, D_K)
            .transpose(3, 0, 2, 1)).astype(BF16)
        kt[:, NC - 1, :, MPC - 1] = 0          # stale slot: score -> 0
        # V -> (128, 32, 4, 128): [p, t, h, d] = V[t*128+p, h, d]
        v = np.ascontiguousarray(
            v_seq[:, hs, :].reshape(NT, 128, HPC, D_K)
            .transpose(1, 0, 2, 3)).astype(BF16)
        v[127, NT - 1, :, :] = 0               # stale slot: A@V term -> 0
        in_maps.append({
            "xt": xt,
            "wq": wproj_layout(W_Q[sl, :]),
            "wk": wproj_layout(W_K[sl, :]),
            "wv": wproj_layout(W_V[sl, :]),
            "wo": np.ascontiguousarray(wo).astype(BF16),
            "kt": kt.reshape(128, NC * HPC * MPC),
            "v": v.reshape(128, NT * HPC * D_K),
        })
    return in_maps


def kernel(x, seq_idx, current_pos, k_cache, v_cache, W_Q, W_K, W_V, W_O):
    x = np.asarray(x, dtype=np.float32)
    k_cache = np.asarray(k_cache)
    v_cache = np.asarray(v_cache)
    W_Q = np.asarray(W_Q, dtype=np.float32)
    W_K = np.asarray(W_K, dtype=np.float32)
    W_V = np.asarray(W_V, dtype=np.float32)
    W_O = np.asarray(W_O, dtype=np.float32)
    seq = int(np.asarray(seq_idx))
    pos = int(np.asarray(current_pos))

    if pos != S - 1 or x.size != D_MODEL or k_cache.shape[1:] != (S, N_HEADS, D_K):
        return _numpy_reference(x, seq, pos, k_cache, v_cache, W_Q, W_K, W_V, W_O)

    from concourse.bass_utils import run_bass_kernel_spmd

    nc = _build_program()
    in_maps = _make_in_maps(x, seq, k_cache, v_cache, W_Q, W_K, W_V, W_O)

    last_err = None
    for _attempt in range(3):
        try:
            res = run_bass_kernel_spmd(nc, in_maps, core_ids=list(range(N_CORES)))
            break
        except Exception as e:          # transient NRT device errors
            last_err = e
    else:
        raise last_err

    y = np.zeros(D_MODEL, dtype=np.float32)
    for c in range(N_CORES):
        y += res.results[c]["out"].reshape(D_MODEL)
    return y.reshape(1, 1, D_MODEL)


# revision 34
# speedup vs baseline: 1.1926x; 1.1926x over previous
"""Trainium2 Bass kernel for single-token decode attention (NaiveAttention).

Math (per reference):
  q = x @ W_Q.T ; k_new = x @ W_K.T ; v_new = x @ W_V.T        (each (32, 128))
  k_cache[seq, pos] = k_new ; v_cache[seq, pos] = v_new
  K = k_cache[seq, :pos+1] ; V = v_cache[seq, :pos+1]
  scores = (q . K) / sqrt(128) ; attn = softmax(scores)
  out = (attn . V) @ W_O.T                                     ((1, 1, 4096))

Sharding: tensor-parallel over heads. 8 cores x 4 heads. W_Q/W_K/W_V are
sharded column-wise (after transpose), W_O row-wise; each core computes a
partial (4096,) output vector and the host sums the 8 partials.

The kernel is DMA-bandwidth-bound: per core it must stream 4 weight shards
(W_Q/W_K/W_V/W_O) + its 4 heads of K and V, all exactly once. Everything is
cast to bf16 on the host (24 MiB/core instead of 48 MiB fp32; measured
~360 GB/s per-core HBM => ~70 us floor) and laid out in DRAM to match the
SBUF destination exactly, so every DMA is fully contiguous. All large
matmuls run bf16 (1 PE cycle/column, fp32 PSUM accumulation).

Scores are computed directly in column form: per (head, 128-seq tile) the
K tile (dk=128 partitions x 128 seq) is the stationary operand and the q
column is the moving operand, so exp() on the scalar engine writes attn
columns straight into SBUF (p_all) with no PE transposes or DVE copies.
A@V is head-batched: lhsT = p(128s x 4heads), rhs = [V_h0|..|V_h3]
(128s x 512); the diagonal 128-blocks of the (4,512) result are the
per-head outputs. The cache slot at s = 4095 is stale: the host zeroes
that K column and V row, so its prob is exactly exp(0) = 1 (subtracted
from the softmax denominator) and its A@V term is exactly 0; the true
k_new/v_new contribution enters as a rank-1 PSUM update plus exp(q.k_new)
on the denominator.
"""

import sys

if "/opt/trn_rl_repo" not in sys.path:
    sys.path.insert(0, "/opt/trn_rl_repo")

import numpy as np
import ml_dtypes

BF16 = ml_dtypes.bfloat16

D_MODEL = 4096
N_HEADS = 32
D_K = 128
S = 4096          # pos + 1 for the compiled fast path
N_CORES = 8
HPC = N_HEADS // N_CORES          # heads per core = 4
MPC = HPC * D_K                   # model dims per core = 512
NT = S // 128                     # 32 seq tiles
NC = S // 512                     # 8 512-wide chunks
INV_SQRT_DK = 1.0 / float(np.sqrt(D_K))

_CACHE = {}


def _build_program():
    """Build + compile the per-core Bass program (identical on all cores)."""
    if "nc" in _CACHE:
        return _CACHE["nc"]

    from concourse import bacc, mybir
    import concourse.tile as tile
    from concourse.masks import make_identity

    f32 = mybir.dt.float32
    bf16 = mybir.dt.bfloat16
    AF = mybir.ActivationFunctionType
    ALU = mybir.AluOpType
    AX = mybir.AxisListType

    nc = bacc.Bacc("TRN2", target_bir_lowering=False, debug=False,
                   num_devices=N_CORES)

    xt_d = nc.dram_tensor("xt", [128, NT], bf16, kind="ExternalInput")
    wq_d = nc.dram_tensor("wq", [128, NT * MPC], bf16, kind="ExternalInput")
    wk_d = nc.dram_tensor("wk", [128, NT * MPC], bf16, kind="ExternalInput")
    wv_d = nc.dram_tensor("wv", [128, NT * MPC], bf16, kind="ExternalInput")
    wo_d = nc.dram_tensor("wo", [128, NC * HPC * MPC], bf16,
                          kind="ExternalInput")          # (128, 8*4*512)
    kt_d = nc.dram_tensor("kt", [128, NC * HPC * MPC], bf16,
                          kind="ExternalInput")          # (128, 8*4*512)
    v_d = nc.dram_tensor("v", [128, NT * HPC * D_K], bf16,
                         kind="ExternalInput")           # (128, 32*4*128)
    out_d = nc.dram_tensor("out", [1, D_MODEL], f32, kind="ExternalOutput")

    with tile.TileContext(nc) as tc:
        with (
            tc.tile_pool(name="singles", bufs=1) as singles,
            tc.tile_pool(name="wpool", bufs=4) as wpool,
            tc.tile_pool(name="projp", bufs=2, space="PSUM") as projp,
            tc.tile_pool(name="scolp", bufs=3, space="PSUM") as scolp,
            tc.tile_pool(name="tp", bufs=2, space="PSUM") as tp,
            tc.tile_pool(name="av4p", bufs=1, space="PSUM") as av4p,
        ):
            # ---- constants / persistent tiles ----
            xt = singles.tile([128, NT], bf16, tag="xt")
            nc.sync.dma_start(xt[:], xt_d.ap())
            ident = singles.tile([128, 128], f32, tag="ident")
            make_identity(nc, ident[:])
            ones_col = singles.tile([128, 1], f32, tag="ones_col")
            nc.vector.memset(ones_col[:], 1.0)

            ktile = singles.tile([128, NC, HPC, MPC], bf16, tag="ktile")
            v4 = singles.tile([128, NT * HPC * D_K], bf16, tag="v4")
            p_all = singles.tile([128, HPC, NT], bf16, tag="p_all")
            qsb = singles.tile([128, HPC], bf16, tag="qsb")
            ksb = singles.tile([128, HPC], bf16, tag="ksb")
            qrow = singles.tile([1, MPC], f32, tag="qrow")
            krow = singles.tile([1, MPC], f32, tag="krow")
            vrow = singles.tile([1, MPC], bf16, tag="vrow")
            rsum = singles.tile([128, HPC], f32, tag="rsum")
            setot = singles.tile([1, HPC], f32, tag="setot")
            recrow = singles.tile([1, HPC], f32, tag="recrow")
            rec4 = singles.tile([HPC, 1], f32, tag="rec4")
            p49f = singles.tile([1, HPC], f32, tag="p49f")
            p49b = singles.tile([1, HPC], bf16, tag="p49b")
            av4n = singles.tile([HPC, MPC], f32, tag="av4n")
            avn = singles.tile([128, HPC], bf16, tag="avn")
            orow = singles.tile([1, D_MODEL], f32, tag="orow")
            wv_tiles = [singles.tile([128, 8, MPC], bf16, name=f"wvt{b}",
                                     tag=f"wvt{b}") for b in range(4)]
            wo_tiles = [singles.tile([128, 2, HPC, MPC], bf16, name=f"wot{b}",
                                     tag=f"wot{b}") for b in range(4)]

            def emit_proj(w_dram, tiles=None):
                """acc(1,512) = x^T @ W_chunk, W streamed as 4 x 1MiB.

                tiles: dedicated SBUF tiles (DMA never throttled by PE
                consumption); falls back to the shared wpool.
                """
                w_ap = w_dram.ap().rearrange("p (b c m) -> b p c m", b=4, c=8)
                acc = projp.tile([1, MPC], f32, tag="projp")
                wts = []
                for b in range(4):
                    wt = (tiles[b] if tiles is not None
                          else wpool.tile([128, 8, MPC], bf16, tag="wt"))
                    nc.sync.dma_start(wt[:], w_ap[b])
                    wts.append(wt)
                for b in range(4):
                    for c in range(8):
                        t = b * 8 + c
                        nc.tensor.matmul(
                            acc[:], xt[:, t:t + 1], wts[b][:, c, :],
                            start=(t == 0), stop=(t == NT - 1),
                            skip_group_check=True)
                return acc

            def row_to_cols(row_sb, dst_sb):
                """(1,512) f32 row -> (128,4) bf16 columns via 4 transposes."""
                t4 = tp.tile([128, HPC], f32, tag="tp")
                for i in range(HPC):
                    nc.tensor.matmul(t4[:, i:i + 1],
                                     row_sb[0:1, i * 128:(i + 1) * 128],
                                     ident[0:1, 0:1], is_transpose=True,
                                     skip_group_check=True)
                nc.vector.tensor_copy(dst_sb[:], t4[:])

            # ---- phase 1: W_Q stream -> q; q -> columns ----
            q_acc = emit_proj(wq_d)
            nc.vector.tensor_copy(qrow[:], q_acc[:])
            row_to_cols(qrow, qsb)

            # ---- phase 2: K stream (8 x 0.5MiB, chunk-major) + scores ----
            # per (seq-tile, head): stationary = K tile (dk x 128s), moving =
            # q column -> score COLUMN in PSUM; exp writes p_all directly.
            kt_ap = kt_d.ap().rearrange("p (c h m) -> c p h m", c=NC, h=HPC)
            for c in range(NC):
                nc.sync.dma_start(ktile[:, c, :, :], kt_ap[c])

            for c in range(NC):
                for j in range(4):
                    t = 4 * c + j
                    scol = scolp.tile([128, HPC], f32, tag="scolp")
                    for g in range(HPC):
                        nc.tensor.matmul(
                            scol[:, g:g + 1],
                            ktile[:, c, g, j * 128:(j + 1) * 128],
                            qsb[:, g:g + 1], skip_group_check=True)
                    nc.scalar.activation(p_all[:, :, t], scol[:], AF.Exp,
                                         scale=INV_SQRT_DK)

            # softmax denominator: sum p over seq (reduce tiles, then a
            # ones-matmul over partitions). The stale s=4095 slot contributes
            # exactly exp(0) = 1 (host zeroed that K column): subtract it.
            nc.vector.tensor_reduce(rsum[:], p_all[:], axis=AX.X, op=ALU.add)
            serow = tp.tile([1, HPC], f32, tag="tp")
            nc.tensor.matmul(serow[:], ones_col[:], rsum[:],
                             skip_group_check=True)

            # ---- phase 4: W_K / W_V streams -> k_new, v_new ----
            k_acc = emit_proj(wk_d)
            nc.vector.tensor_copy(krow[:], k_acc[:])
            row_to_cols(krow, ksb)

            # p4095_h = exp(q_h . k_new_h / sqrt(dk)); sumexp finalization
            sc4 = tp.tile([1, HPC], f32, tag="tp")
            for h in range(HPC):
                nc.tensor.matmul(sc4[:, h:h + 1], ksb[:, h:h + 1],
                                 qsb[:, h:h + 1], skip_group_check=True)
            nc.scalar.activation(p49f[:], sc4[:], AF.Exp, scale=INV_SQRT_DK)
            nc.vector.tensor_copy(p49b[:], p49f[:])
            nc.vector.tensor_scalar_add(setot[:], p49f[:], -1.0)
            nc.vector.tensor_add(setot[:], setot[:], serow[:])
            nc.vector.reciprocal(recrow[:], setot[:])

            v_acc = emit_proj(wv_d, tiles=wv_tiles)
            nc.vector.tensor_copy(vrow[:], v_acc[:])

            # rec (1,4) row -> (4,1) per-partition scalars
            rect = tp.tile([HPC, 1], f32, tag="tp")
            nc.tensor.matmul(rect[:], recrow[:], ident[0:1, 0:1],
                             is_transpose=True, skip_group_check=True)
            nc.vector.tensor_copy(rec4[:], rect[:])

            # ---- phase 5: V stream (4 x 1MiB) + head-batched A@V ----
            # emitted after the W_V projection so the A@V matmuls track the
            # V DMA and the rank-1 closer can follow immediately
            # last piece split in two so its completion semaphore (which
            # gates the final A@V matmuls) fires earlier
            v_ap = v_d.ap().rearrange("p (b r) -> b p r", b=8)
            for b in range(8):
                if b < 6 and b % 2 == 0:
                    nc.sync.dma_start(
                        v4[:, b * 2048:(b + 2) * 2048],
                        v_d.ap()[:, b * 2048:(b + 2) * 2048])
                elif b >= 6:
                    nc.sync.dma_start(v4[:, b * 2048:(b + 1) * 2048], v_ap[b])

            av4 = av4p.tile([HPC, HPC * D_K], f32, tag="av4")
            for t in range(NT):
                nc.tensor.matmul(av4[:], p_all[:, :, t],
                                 v4[:, t * 512:(t + 1) * 512],
                                 start=(t == 0), stop=False,
                                 skip_group_check=True)
            # av4[g, (h,d)] += p4095_g * v_new_h[d]; diagonal g==h is the
            # true s = S-1 contribution; closes the A@V accumulation group
            nc.tensor.matmul(av4[:], p49b[:], vrow[:],
                             start=False, stop=True, skip_group_check=True)

            # ---- phase 6: normalize + diagonal extraction ----
            nc.vector.tensor_scalar_mul(av4n[:], av4[:], rec4[:, 0:1])
            for g in range(HPC):
                avt = tp.tile([128, HPC], f32, tag="tp")
                nc.tensor.matmul(avt[:], av4n[0:HPC, g * 128:(g + 1) * 128],
                                 ident[0:HPC, 0:HPC], is_transpose=True,
                                 skip_group_check=True)
                nc.vector.tensor_copy(avn[:, g:g + 1], avt[:, g:g + 1])

            # ---- phase 7: W_O stream + partial output ----
            # all W_O piece DMAs enqueue before the output-chunk DMAs so the
            # sync queue never blocks the stream on the compute chain
            wo_ap = wo_d.ap().rearrange("p (b j h m) -> b p j h m",
                                        b=4, j=2, h=HPC)
            for b in range(3):
                nc.sync.dma_start(wo_tiles[b][:], wo_ap[b])
            for jj in range(2):     # last piece split: earlier completion
                nc.sync.dma_start(wo_tiles[3][:, jj], wo_ap[3][:, jj])
            for b in range(4):
                wt = wo_tiles[b]
                for jj in range(2):
                    jc = b * 2 + jj
                    wo_ps = projp.tile([1, MPC], f32, tag="projp")
                    for h in range(HPC):
                        nc.tensor.matmul(
                            wo_ps[:], avn[:, h:h + 1], wt[:, jj, h, :],
                            start=(h == 0), stop=(h == HPC - 1),
                            skip_group_check=True)
                    nc.vector.tensor_copy(orow[:, jc * 512:(jc + 1) * 512],
                                          wo_ps[:])
            # single 16 KiB result DMA: avoids 8 serial sync-engine issues
            nc.sync.dma_start(out_d.ap(), orow[:])

    nc.compile()
    _CACHE["nc"] = nc
    return nc


def _numpy_reference(x, seq, pos, k_cache, v_cache, W_Q, W_K, W_V, W_O):
    """Fallback for shapes the compiled program doesn't cover."""
    xf = x.reshape(-1).astype(np.float32)
    q = (W_Q @ xf).reshape(N_HEADS, D_K)
    k_new = (W_K @ xf).reshape(N_HEADS, D_K)
    v_new = (W_V @ xf).reshape(N_HEADS, D_K)
    K = np.array(k_cache[seq, :pos + 1], dtype=np.float32)
    V = np.array(v_cache[seq, :pos + 1], dtype=np.float32)
    K[pos] = k_new
    V[pos] = v_new
    scores = np.einsum("hd,shd->hs", q, K) / np.float32(np.sqrt(D_K))
    scores -= scores.max(axis=-1, keepdims=True)
    e = np.exp(scores)
    attn = e / e.sum(axis=-1, keepdims=True)
    out = np.einsum("hs,shd->hd", attn, V).reshape(-1)
    return (W_O @ out).reshape(1, 1, D_MODEL).astype(np.float32)


def _make_in_maps(x, seq, k_cache, v_cache, W_Q, W_K, W_V, W_O):
    xt = np.ascontiguousarray(x.reshape(NT, 128).T).astype(BF16)
    k_seq = np.asarray(k_cache[seq], dtype=np.float32)   # (S, H, dk)
    v_seq = np.asarray(v_cache[seq], dtype=np.float32)

    def wproj_layout(W_shard):
        # (512, 4096) -> (128, 32*512): [p, t, m] = W_shard[m, t*128+p]
        return np.ascontiguousarray(
            W_shard.T.reshape(NT, 128, MPC).transpose(1, 0, 2)
            .reshape(128, NT * MPC)).astype(BF16)

    in_maps = []
    for c in range(N_CORES):
        sl = slice(c * MPC, (c + 1) * MPC)
        hs = slice(c * HPC, (c + 1) * HPC)
        # W_O[:, sl] -> (128, 8, 4, 512): [p, jc, h, m] = W_O[jc*512+m, sl0+h*128+p]
        wo = (W_O[:, sl].reshape(NC, MPC, HPC, 128)
              .transpose(3, 0, 2, 1).reshape(128, NC * HPC * MPC))
        # K -> (128, 8, 4, 512): [d, c8, h, j] = K[c8*512+j, h, d]
        kt = np.ascontiguousarray(
            k_seq[:, hs, :].reshape(NC, MPC, HPC, D_K)
            .transpose(3, 0, 2, 1)).astype(BF16)
        kt[:, NC - 1, :, MPC - 1] = 0          # stale slot: score -> 0
        # V -> (128, 32, 4, 128): [p, t, h, d] = V[t*128+p, h, d]
        v = np.ascontiguousarray(
            v_seq[:, hs, :].reshape(NT, 128, HPC, D_K)
            .transpose(1, 0, 2, 3)).astype(BF16)
        v[127, NT - 1, :, :] = 0               # stale slot: A@V term -> 0
        in_maps.append({
            "xt": xt,
            "wq": wproj_layout(W_Q[sl, :]),
            "wk": wproj_layout(W_K[sl, :]),
            "wv": wproj_layout(W_V[sl, :]),
            "wo": np.ascontiguousarray(wo).astype(BF16),
            "kt": kt.reshape(128, NC * HPC * MPC),
            "v": v.reshape(128, NT * HPC * D_K),
        })
    return in_maps


def kernel(x, seq_idx, current_pos, k_cache, v_cache, W_Q, W_K, W_V, W_O):
    x = np.asarray(x, dtype=np.float32)
    k_cache = np.asarray(k_cache)
    v_cache = np.asarray(v_cache)
    W_Q = np.asarray(W_Q, dtype=np.float32)
    W_K = np.asarray(W_K, dtype=np.float32)
    W_V = np.asarray(W_V, dtype=np.float32)
    W_O = np.asarray(W_O, dtype=np.float32)
    seq = int(np.asarray(seq_idx))
    pos = int(np.asarray(current_pos))

    if pos != S - 1 or x.size != D_MODEL or k_cache.shape[1:] != (S, N_HEADS, D_K):
        return _numpy_reference(x, seq, pos, k_cache, v_cache, W_Q, W_K, W_V, W_O)

    from concourse.bass_utils import run_bass_kernel_spmd

    nc = _build_program()
    in_maps = _make_in_maps(x, seq, k_cache, v_cache, W_Q, W_K, W_V, W_O)

    last_err = None
    for _attempt in range(3):
        try:
            res = run_bass_kernel_spmd(nc, in_maps, core_ids=list(range(N_CORES)))
            break
        except Exception as e:          # transient NRT device errors
            last_err = e
    else:
        raise last_err

    y = np.zeros(D_MODEL, dtype=np.float32)
    for c in range(N_CORES):
        y += res.results[c]["out"].reshape(D_MODEL)
    return y.reshape(1, 1, D_MODEL)
